# revision 2
# baseline (speedup 1.0000x reference)
"""Trainium2 Bass kernel for a 3-layer bidirectional GRU + dense sigmoid head.

Problem: B=256, T=512, D=256, H=128 (Keras reset_after=True, gate order z,r,h).
Sharding: data-parallel over batch, 32 examples per core on 8 NeuronCores.

Per-core design v2 (gate-partition layout, everything [128(h-dim), cols]).

The scan's per-step critical cycle is shortened by splitting the state
update  h_t = z*h + (1-z)*hh  into
    p1_t = z_t * h_{t-1}          (ready right after the sigmoid)
    v_t  = (z_t - 1) * hh_t       (ready right after the tanh)
    h_t  = p1_t - v_t             (off the critical path; only gating+output)
and feeding the recurrence matmuls with p1/v separately:
    U h_t = U p1_t + (-U) v_t     (PSUM accumulation, negated copy of U).
The p1 matmuls run ~3 pipeline links before v is ready, so the cycle is
    tanh -> v -> mm(-U v) -> sigmoid -> tt -> arg -> tanh
instead of threading the whole  dd/ee/out  ladder through DVE.

Engines: PE recurrence+xp GEMMs; ACT sigmoid/tanh; DVE tt/arg/v;
Pool (gpsimd) computes p1 and h (SBUF-only ops) to decongest DVE.

PSUM (8 banks): per group parity {zr_f, zr_b, xh} (1 bank each, GRP=8,
double-buffered => 6) + 2 rotating per-step scratch banks (c_f, c_b,
arg_f, arg_b).  z,r share a bank: the first matmul of a group's xp GEMM
uses start=True (whole-bank has_written clear); everything after runs
start=False and the per-element has_written bits give overwrite-then-
accumulate semantics (verified on HW).

The xp GEMMs of group g+1 are emitted interleaved into group g's steps
(PSUM is double-buffered), so the PE never bursts at group boundaries.
"""

from contextlib import ExitStack

import numpy as np

import concourse.bass as bass
from concourse import bacc
import concourse.mybir as mybir
import concourse.tile as tile
from concourse.bass_utils import run_bass_kernel_spmd

H = 128
D_IN = 256
N_CORES = 8
GRP = 8
F32 = mybir.dt.float32
F32R = mybir.dt.float32r
BF16 = mybir.dt.bfloat16
AF = mybir.ActivationFunctionType
ALU = mybir.AluOpType


def _r(ap):
    return ap


def build_kwargs(GRP=8):
    return {"GRP": GRP, "has_bias": True, "has_bhh": False}


def build_gru(nc, B, T, L, GRP=8, has_bias=False, has_bhh=False):
    NG = T // GRP
    assert T % GRP == 0

    CW = L * 2 * 2 * 3 * H          # 4608
    CU = L * 2 * 3 * H              # 2304
    c_u = CW
    c_un = CW + CU                  # negated U copy
    c_wd = CW + 2 * CU
    c_bias = c_wd + 2
    c_bhh = c_bias + CU
    c_ones = c_bhh + L * H
    c_ind2 = c_ones + GRP * B
    c_h0 = c_ind2 + 2 * B
    C = c_h0 + 2 * B
    x = nc.dram_tensor("x", [D_IN, T * B], BF16, kind="ExternalInput")
    wpack = nc.dram_tensor("wpack", [H, C], BF16, kind="ExternalInput")
    y = nc.dram_tensor("y", [1, B], F32, kind="ExternalOutput")

    with tile.TileContext(nc) as tc, ExitStack() as ctx:
        const = ctx.enter_context(tc.tile_pool(name="const", bufs=1))
        rhsp = ctx.enter_context(tc.tile_pool(name="rhsp", bufs=3))
        outp = ctx.enter_context(tc.tile_pool(name="outp", bufs=3))
        stepp = ctx.enter_context(tc.tile_pool(name="stepp", bufs=4))
        psum = ctx.enter_context(tc.tile_pool(name="psum", bufs=2,
                                              space="PSUM"))
        pscr = ctx.enter_context(tc.tile_pool(name="pscr", bufs=2,
                                              space="PSUM"))
        dramp = ctx.enter_context(tc.tile_pool(name="dramp", bufs=1,
                                               space="DRAM"))

        # inter-layer hidden-sequence buffers: [H, dir, T*B]
        seqs = [dramp.tile([H, 2, T * B], BF16, name=f"seq{p}", tag=f"seq{p}")
                for p in "AB"]

        pk = const.tile([H, C], BF16)
        nc.sync.dma_start(out=pk, in_=wpack[:])

        def w_ap(l, d, k, gi):
            c = ((l * 2 + d) * 2 + k) * 3 * H + gi * H
            return pk[:, c:c + H]

        def u_ap(l, d, gi):
            c = c_u + (l * 2 + d) * 3 * H + gi * H
            return pk[:, c:c + H]

        def un_ap(l, d, gi):
            c = c_un + (l * 2 + d) * 3 * H + gi * H
            return pk[:, c:c + H]

        def wd_ap(d):
            return pk[:, c_wd + d:c_wd + d + 1]

        def bias_ap(l, d, gi):
            c = c_bias + (l * 2 + d) * 3 * H + gi * H
            return pk[0:1, c:c + H]

        def bhh_ap(l):
            return pk[0:2, c_bhh + l * H:c_bhh + (l + 1) * H]

        zeros2 = pk[:, c_h0:c_h0 + 2 * B].rearrange("p (d b) -> p d b", d=2)
        ones_sb = pk[0:1, c_ones:c_ones + GRP * B]
        ind2_sb = pk[0:2, c_ind2:c_ind2 + 2 * B]

        xv = x[:].rearrange("(k p) (t b) -> p k t b", k=2, b=B)  # [128,2,T,B]

        def emit_rhs_dma(l, g):
            """One DMA per direction: [H, 2(k), GRP, B] f32r."""
            rhs = []
            for d, dn in ((0, "f"), (1, "b")):
                t_lo = GRP * g if d == 0 else T - GRP * (g + 1)
                rt = rhsp.tile([H, 2, GRP, B], BF16, tag=f"rhs{dn}",
                               name=f"rhs_{dn}_{l}_{g}")
                if l == 0:
                    src = xv[:, :, t_lo:t_lo + GRP, :]
                else:
                    src = seqs[(l - 1) % 2][:].rearrange(
                        "p d (t b) -> p d t b", b=B)[:, :, t_lo:t_lo + GRP, :]
                nc.sync.dma_start(out=rt, in_=src)
                rhs.append(rt)
            return rhs

        def alloc_banks(l, g):
            zr_f = psum.tile([H, 2, GRP, B], F32, tag="zrf",
                             name=f"zrf_{l}_{g}")
            zr_b = psum.tile([H, 2, GRP, B], F32, tag="zrb",
                             name=f"zrb_{l}_{g}")
            xh = psum.tile([H, 2, GRP, B], F32, tag="xh",
                           name=f"xh_{l}_{g}")
            xhs = rhsp.tile([H, 2, GRP, B], BF16, tag="xhs",
                            name=f"xhs_{l}_{g}")
            return zr_f, zr_b, xh, xhs

        def xp_closures(l, banks, rhs):
            """Closures, one matmul each; per-bank first mm has start=True.
            Ends with an ACT bulk-copy of the xh bank to SBUF so the
            per-step arg add is a cheap SBUF*SBUF DVE op."""
            zr_f, zr_b, xh, xhs = banks
            out = []
            for d in (0, 1):
                zr_d = (zr_f, zr_b)[d]
                for gi in range(2):
                    for k in range(2):
                        first = (gi == 0 and k == 0)
                        out.append(lambda l=l, d=d, gi=gi, k=k, zr_d=zr_d, \
                                   first=first: nc.tensor.matmul(
                            zr_d[:, gi, :, :], _r(w_ap(l, d, k, gi)),
                            _r(rhs[d][:, k, :, :]), start=first, stop=False,
                            skip_group_check=True))
            for d in (0, 1):
                for k in range(2):
                    out.append(lambda l=l, d=d, k=k, xh=xh:
                               nc.tensor.matmul(
                        xh[:, d, :, :], _r(w_ap(l, d, k, 2)),
                        _r(rhs[d][:, k, :, :]), start=(d == 0 and k == 0),
                        stop=False, skip_group_check=True))
            return out

        def xh_copy_closure(banks):
            zr_f, zr_b, xh, xhs = banks
            return lambda: nc.scalar.copy(xhs, xh)

        # bias via ones-row GEMMs: z,r bias per (l,d) spans the two gate
        # regions of a zr bank; emit as two single-gate GEMMs instead.
        def xp_bias_closures(l, banks):
            zr_f, zr_b, xh, xhs = banks
            out = []
            for d in (0, 1):
                zr_d = (zr_f, zr_b)[d]
                for gi in range(2):
                    out.append(lambda l=l, d=d, gi=gi, zr_d=zr_d:
                               nc.tensor.matmul(
                        zr_d[:, gi, :, :], _r(bias_ap(l, d, gi)), _r(ones_g),
                        start=False, stop=False, skip_group_check=True))
                out.append(lambda l=l, d=d: nc.tensor.matmul(
                    xh[:, d, :, :], _r(bias_ap(l, d, 2)), _r(ones_g),
                    start=False, stop=False, skip_group_check=True))
            return out

        ones_g = ones_sb.rearrange("p (t b) -> p t b", b=B)  # [1, GRP, B]

        last_outbuf = None

        for l in range(L):
            # ---- layer prolog: group 0 rhs + xp GEMMs ----
            rhs = emit_rhs_dma(l, 0)
            banks = alloc_banks(l, 0)
            for fn in xp_closures(l, banks, rhs):
                fn()
            if has_bias:
                for fn in xp_bias_closures(l, banks):
                    fn()
            xh_copy_closure(banks)()

            # previous-step state: per-dir (p1, v, h) APs; zeros at start
            prev_p1 = [zeros2[:, 0, :], zeros2[:, 1, :]]
            prev_v = [zeros2[:, 0, :], zeros2[:, 1, :]]
            prev_h = (zeros2[:, 0, :], zeros2[:, 1, :])
            outbuf = None
            prev_scr = None

            for g in range(NG):
                pending = []
                if g + 1 < NG:
                    rhs_n = emit_rhs_dma(l, g + 1)
                    banks_n = alloc_banks(l, g + 1)
                    pending = xp_closures(l, banks_n, rhs_n)
                    if has_bias:
                        pending += xp_bias_closures(l, banks_n)
                    pending.append(xh_copy_closure(banks_n))

                prev_outbuf = outbuf
                outbuf = outp.tile([H, 2, GRP, B], BF16, tag="outbuf",
                                   name=f"outbuf_{l}_{g}")
                zr_f, zr_b, xh, xhs = banks

                for tl in range(GRP):
                    cf, cb = tl, GRP - 1 - tl
                    cols = (cf, cb)

                    # -- (a) p1 matmuls for this step (+ c scratch init) --
                    scr = pscr.tile([H, 2, B], F32, tag="scr",
                                    name=f"scr_{l}_{g}_{tl}")
                    for d in (0, 1):
                        nc.tensor.matmul(scr[:, d, :], _r(u_ap(l, d, 2)),
                                         _r(prev_p1[d]),
                                         start=(d == 0), stop=False,
                                         skip_group_check=True)
                    if has_bhh:
                        nc.tensor.matmul(scr[:, 0:2, :], _r(bhh_ap(l)),
                                         _r(ind2_sb), start=False, stop=False,
                                         skip_group_check=True)
                    for d in (0, 1):
                        zr_d = (zr_f, zr_b)[d]
                        for gi in (0, 1):
                            nc.tensor.matmul(zr_d[:, gi, cols[d], :],
                                             _r(u_ap(l, d, gi)),
                                             _r(prev_p1[d]),
                                             start=False, stop=False,
                                             skip_group_check=True)
                    # -- (b) v matmuls for this step (negated U); z,r first:
                    # the sigmoid (on the critical cycle) waits on them,
                    # while the c matmuls are hidden behind the sigmoid --
                    for d in (0, 1):
                        zr_d = (zr_f, zr_b)[d]
                        for gi in (0, 1):
                            nc.tensor.matmul(zr_d[:, gi, cols[d], :],
                                             _r(un_ap(l, d, gi)),
                                             _r(prev_v[d]),
                                             start=False, stop=True,
                                             skip_group_check=True)
                    for d in (0, 1):
                        nc.tensor.matmul(scr[:, d, :], _r(un_ap(l, d, 2)),
                                         _r(prev_v[d]), start=False,
                                         stop=True, skip_group_check=True)
                    # -- (c) interleave next group's xp GEMMs --
                    for _ in range(3):
                        if pending:
                            pending.pop(0)()

                    # -- per-step per-direction tiles (separate tags: tile-
                    # granular dep tracking must not couple the f/b chains) --
                    nm = f"{l}_{g}_{tl}"
                    zro = [stepp.tile([H, 2, B], BF16, tag=f"zro{d}",
                                      name=f"zro{d}_{nm}") for d in (0, 1)]
                    tt = [stepp.tile([H, B], BF16, tag=f"tt{d}",
                                     name=f"tt{d}_{nm}") for d in (0, 1)]
                    arg = [stepp.tile([H, B], BF16, tag=f"arg{d}",
                                      name=f"arg{d}_{nm}") for d in (0, 1)]
                    hh = [stepp.tile([H, B], BF16, tag=f"hh{d}",
                                     name=f"hh{d}_{nm}") for d in (0, 1)]
                    p1 = [stepp.tile([H, B], BF16, tag=f"p1{d}",
                                     name=f"p1{d}_{nm}") for d in (0, 1)]
                    v = [stepp.tile([H, B], BF16, tag=f"v{d}",
                                    name=f"v{d}_{nm}") for d in (0, 1)]

                    # -- (d) sigmoids --
                    for d in (0, 1):
                        zr_d = (zr_f, zr_b)[d]
                        nc.scalar.activation(zro[d], zr_d[:, :, cols[d], :],
                                             AF.Sigmoid)
                    # -- (e) p1 = z * h_prev  (Pool) --
                    for d in (0, 1):
                        nc.gpsimd.tensor_mul(p1[d], zro[d][:, 0, :],
                                             prev_h[d])
                    # -- (f) tt = c * r ; arg = tt + xh  (DVE)
                    # emission order = scheduler priority: keep each dir's
                    # tt->arg adjacent so arg_f never queues behind tt_b --
                    for d in (0, 1):
                        nc.vector.tensor_mul(tt[d], scr[:, d, :],
                                             zro[d][:, 1, :])
                        nc.vector.tensor_add(arg[d], tt[d],
                                             xhs[:, d, cols[d], :])
                    # -- (g) tanh --
                    for d in (0, 1):
                        nc.scalar.activation(hh[d], arg[d], AF.Tanh)
                    # -- (h) v = (z - 1) * hh  (DVE STT) --
                    for d in (0, 1):
                        nc.vector.scalar_tensor_tensor(
                            v[d], zro[d][:, 0, :], 1.0, hh[d],
                            ALU.subtract, ALU.mult)
                    # -- (i) h = p1 - v  (Pool) --
                    for d in (0, 1):
                        nc.gpsimd.tensor_sub(outbuf[:, d, cols[d], :],
                                             p1[d], v[d])

                    prev_p1, prev_v = p1, v
                    prev_h = (outbuf[:, 0, cf, :], outbuf[:, 1, cb, :])

                while pending:
                    pending.pop(0)()

                # ---- store the group's hidden states (layers 0..L-2) ----
                if l < L - 1:
                    sq = seqs[l % 2][:].rearrange("p d (t b) -> p d t b", b=B)
                    nc.sync.dma_start(
                        out=sq[:, 0, GRP * g:GRP * (g + 1), :],
                        in_=outbuf[:, 0, :, :])
                    t_lo_b = T - GRP * (g + 1)
                    nc.sync.dma_start(
                        out=sq[:, 1, t_lo_b:t_lo_b + GRP, :],
                        in_=outbuf[:, 1, :, :])

                if g + 1 < NG:
                    banks = banks_n
                # h carry across groups: prev_h already points into outbuf
                prev_h = (outbuf[:, 0, GRP - 1, :], outbuf[:, 1, 0, :])

            last_outbuf = outbuf

        # ---- dense head on the final states ----
        py = pscr.tile([1, B], F32, tag="scr", name="py")
        nc.tensor.matmul(py, _r(wd_ap(0)), _r(last_outbuf[:, 0, GRP - 1, :]),
                         start=True, stop=False, skip_group_check=True)
        nc.tensor.matmul(py, _r(wd_ap(1)), _r(last_outbuf[:, 1, 0, :]),
                         start=False, stop=True, skip_group_check=True)
        y_sb = const.tile([1, B], F32)
        nc.scalar.activation(y_sb, py, AF.Sigmoid)
        nc.sync.dma_start(out=y[:], in_=y_sb)


def _prep_host(Ws, Us, bs, Wd, L, GRP, B_loc):
    """Pack all replicated weights into one [128, C] array (single DMA)."""
    Ws = np.asarray(Ws, np.float32)
    Us = np.asarray(Us, np.float32)
    bs = np.asarray(bs, np.float32)
    Wd = np.asarray(Wd, np.float32)
    has_bias = bool(np.any(bs != 0))
    has_bhh = bool(np.any(bs[:, :, 1, 2 * H:] != 0))
    CW = L * 2 * 2 * 3 * H
    CU = L * 2 * 3 * H
    GRPB = GRP * B_loc
    C = CW + 2 * CU + 2 + CU + L * H + GRPB + 4 * B_loc
    pack = np.zeros((H, C), np.float32)
    c_bias = CW + 2 * CU + 2
    c_bhh = c_bias + CU
    c_ones = c_bhh + L * H
    pack[0, c_ones:c_ones + GRPB] = 1.0
    pack[0, c_ones + GRPB:c_ones + GRPB + B_loc] = 1.0        # ind2 row 0
    pack[1, c_ones + GRPB + B_loc:c_ones + GRPB + 2 * B_loc] = 1.0
    pack[:, :CW] = (Ws.reshape(L, 2, 2, H, 3 * H)
                    .transpose(3, 0, 1, 2, 4).reshape(H, CW))
    pack[:, CW:CW + CU] = Us.transpose(2, 0, 1, 3).reshape(H, CU)
    pack[:, CW + CU:CW + 2 * CU] = -pack[:, CW:CW + CU]
    pack[:, CW + 2 * CU] = Wd[0:H, 0]
    pack[:, CW + 2 * CU + 1] = Wd[H:2 * H, 0]
    if has_bias:
        bsum = bs[:, :, 0, :].copy()               # b_i everywhere
        bsum[:, :, :2 * H] += bs[:, :, 1, :2 * H]  # + b_h on z,r
        pack[0, c_bias:c_bias + CU] = bsum.reshape(-1)
    if has_bhh:
        pack[0:2, c_bhh:c_bhh + L * H] = np.transpose(
            bs[:, :, 1, 2 * H:], (1, 0, 2)).reshape(2, L * H)
    import ml_dtypes
    return {"wpack": pack.astype(ml_dtypes.bfloat16)}, has_bias, has_bhh


def run_gru(x, Ws, Us, bs, Wd, bd, n_cores=N_CORES, L=3, GRP=8, trace=False):
    x = np.ascontiguousarray(np.asarray(x, np.float32))
    B_full, T, _ = x.shape
    B_loc = B_full // n_cores
    common, has_bias, has_bhh = _prep_host(Ws, Us, bs, Wd, L, GRP, B_loc)

    nc = bacc.Bacc()
    build_gru(nc, B_loc, T, L, GRP, has_bias, has_bhh)
    nc.compile()

    in_maps = []
    for c in range(n_cores):
        m = dict(common)
        xs = x[c * B_loc:(c + 1) * B_loc]          # [B_loc, T, D]
        import ml_dtypes
        m["x"] = np.ascontiguousarray(
            xs.transpose(2, 1, 0).reshape(D_IN, T * B_loc)).astype(
                ml_dtypes.bfloat16)
        in_maps.append(m)

    res = run_bass_kernel_spmd(nc, in_maps, core_ids=list(range(n_cores)),
                               trace=trace)
    parts = [res.results[c]["y"][0] for c in range(n_cores)]
    out = np.concatenate(parts).reshape(B_full, 1).astype(np.float32)
    return out, res


def kernel(x, Ws, Us, bs, Wd, bd):
    bd = np.asarray(bd, np.float32).reshape(-1)
    out, _ = run_gru(x, Ws, Us, bs, Wd, bd)
    if np.any(bd != 0):
        p = np.clip(np.float64(out), 1e-12, 1 - 1e-12)
        out = (1.0 / (1.0 + np.exp(-(np.log(p / (1 - p)) + bd[0]))))
    return np.asarray(out, np.float32)
